# revision 7
# baseline (speedup 1.0000x reference)
"""Causal MQA self-attention (RoPE + RMS-norm on q/k) on 8 TRN2 NeuronCores.

Sharding: core c -> (batch b = c//4, head-group g = c%4 of 4 heads).
Each core computes, for its batch and its 4 heads:
  q/k/v projections -> RoPE -> RMS-norm -> causal attention -> partial
  output projection out_part = attn_out_g @ wo[:, g].T  (shape [S, HID]).
Host sums the 4 per-group partials of each batch (row-parallel matmul
unshard) and stacks the 2 batches.

All heavy matmuls run in float32r (full PE rate at moving-dim >= 256,
~1.6e-4 scale-relative accuracy). Attention probabilities and V run in
bf16 with fp32 PSUM accumulation; the softmax denominator comes free via
a ones-column appended to V. No max-subtraction is needed: post-RMS-norm
|q|=|k|=sqrt(D), so scores <= sqrt(D) ~ 11.3 and exp stays in range.
"""

import numpy as np

import concourse.bass as bass
import concourse.mybir as mybir
import concourse.tile as tile
from concourse import bacc
from concourse.bass_utils import run_bass_kernel_spmd
from concourse.masks import make_identity, make_upper_triangular

# problem dims (hardcoded per contract)
B, S, HID, H, D = 2, 2048, 2048, 16, 128
NCORES = 8
GROUPS = 4              # head-groups = cores per batch
HG = H // GROUPS        # heads per core
DG = HG * D             # 512 projected q dims per core
NT = S // 128           # 16 sequence tiles
HT = HID // 128         # 16 hidden tiles
NQC = 4                 # q chunks of 512 columns
EPS = 1.1920928955078125e-07
ISD = 1.0 / float(np.sqrt(D))

f32 = mybir.dt.float32
f32r = mybir.dt.float32r
bf16 = mybir.dt.bfloat16

TRACE = False           # test harness may flip this for NTFF profiling
PH1_TILES = NT          # bisect knob: how many s-tiles to emit in phase 1
PH2_CHUNKS = NQC        # bisect knob: how many q-chunks to emit in phase 2
EMIT_SCORES = True      # bisect knob
EMIT_PV = True          # bisect knob
EMIT_WO = True          # bisect knob
LAST = {}               # last BassKernelResults, for the test harness

_compiled = None


def _emit(nc, xT, wqT, wkvT, woT, csx, snx, out):
    mult = mybir.AluOpType.mult
    add = mybir.AluOpType.add
    Sqrt = mybir.ActivationFunctionType.Sqrt
    Exp = mybir.ActivationFunctionType.Exp

    with tile.TileContext(nc) as tc:
        with (
            tc.tile_pool(name="consts", bufs=1) as consts,
            tc.tile_pool(name="bigp", bufs=1) as bigp,
            tc.tile_pool(name="xsp", bufs=3) as xsp,
            tc.tile_pool(name="csp", bufs=1) as csp,
            tc.tile_pool(name="rsp", bufs=2) as rsp,
            tc.tile_pool(name="smp", bufs=2) as smp,
            tc.tile_pool(name="qnp", bufs=2) as qnp,
            tc.tile_pool(name="ptp", bufs=9) as ptp,
            tc.tile_pool(name="otp", bufs=2) as otp,
            tc.tile_pool(name="ocp", bufs=2) as ocp,
            tc.tile_pool(name="pA", bufs=2, space="PSUM") as pA,
            tc.tile_pool(name="pB", bufs=2, space="PSUM") as pB,
            tc.tile_pool(name="pC", bufs=2, space="PSUM") as pC,
        ):
            # ---- constants ----
            ident = consts.tile([128, 128], f32)
            make_identity(nc, ident)
            cmask = consts.tile([128, 128], bf16)  # 1 where k <= q else 0
            make_upper_triangular(nc, cmask, val=1.0, diag=True)
            eps_t = consts.tile([128, 1], f32)
            nc.vector.memset(eps_t, EPS)

            # ---- resident weights / activations ----
            wq_sb = bigp.tile([128, HT, DG], f32r, tag="wq")
            nc.sync.dma_start(wq_sb, wqT.rearrange("(t p) d -> p t d", p=128))
            wkv_sb = bigp.tile([128, HT, 2 * D], f32r, tag="wkv")
            nc.sync.dma_start(wkv_sb, wkvT.rearrange("(t p) d -> p t d", p=128))
            wo_sb = bigp.tile([128, HG, HID], f32r, tag="wo")
            nc.sync.dma_start(wo_sb, woT.rearrange("(h p) n -> p h n", p=128))

            qT_all = bigp.tile([128, HG, S], f32r, tag="qT")   # [d, h, s]
            kT_sb = bigp.tile([128, S], f32r, tag="kT")        # [d, s]
            vv = bigp.tile([128, NT, 132], bf16, tag="vv")     # [s%128, s//128, d+ones]
            nc.vector.memset(vv[:, :, 128:132], 1.0)

            xTr = xT.rearrange("(t p) s -> p t s", p=128)

            def bcast4(src2d, st):
                base = src2d[st * 128:(st + 1) * 128, :]
                return bass.AP(
                    tensor=base.tensor,
                    offset=base.offset,
                    ap=[base.ap[0], [0, HG], base.ap[1]],
                )

            # ================= phase 1: projections + RoPE + RMS-norm =========
            for st in range(PH1_TILES):
                xs0 = xsp.tile([128, HT // 2, 128], f32r, tag="xs")
                nc.sync.dma_start(xs0, xTr[:, 0:HT // 2, st * 128:(st + 1) * 128])
                xs1 = xsp.tile([128, HT // 2, 128], f32r, tag="xs")
                nc.sync.dma_start(xs1, xTr[:, HT // 2:HT, st * 128:(st + 1) * 128])
                xhalves = (xs0, xs1)

                cs_t = csp.tile([128, HG, 128], f32, tag="cs")
                nc.gpsimd.dma_start(cs_t, bcast4(csx, st))
                sn_t = csp.tile([128, HG, 128], f32, tag="sn")
                nc.gpsimd.dma_start(sn_t, bcast4(snx, st))

                qp = pA.tile([128, 2, DG], f32, tag="A")
                for t in range(HT):
                    nc.tensor.matmul(
                        qp[:, 0, :], lhsT=xhalves[t // 8][:, t % 8, :],
                        rhs=wq_sb[:, t, :], start=(t == 0), stop=(t == HT - 1),
                    )
                kvp = pA.tile([128, 2, DG], f32, tag="A")
                for t in range(HT):
                    nc.tensor.matmul(
                        kvp[:, 0, 0:2 * D], lhsT=xhalves[t // 8][:, t % 8, :],
                        rhs=wkv_sb[:, t, :], start=(t == 0), stop=(t == HT - 1),
                    )

                # v -> bf16 tiles (ones column preset)
                nc.vector.tensor_copy(vv[:, st, 0:128], kvp[:, 0, D:2 * D])

                # ---- RoPE + RMS-norm for 4 q heads, batched ----
                qflat = qp[:, 0, :]
                q3 = qflat.rearrange("p (h d) -> p h d", h=HG)
                q4 = qflat.rearrange("p (h t d) -> p h t d", h=HG, t=2)
                rot = rsp.tile([128, DG], f32, tag="rot")
                r4 = rot.rearrange("p (h t d) -> p h t d", h=HG, t=2)
                r3 = rot.rearrange("p (h d) -> p h d", h=HG)
                nc.vector.tensor_copy(r4[:, :, 0, :], q4[:, :, 1, :])
                nc.vector.tensor_scalar_mul(r4[:, :, 1, :], q4[:, :, 0, :], -1.0)
                t1 = rsp.tile([128, DG], f32, tag="t1")
                t3 = t1.rearrange("p (h d) -> p h d", h=HG)
                nc.vector.tensor_mul(t3, q3, cs_t)
                nc.vector.tensor_mul(r3, r3, sn_t)
                nc.vector.tensor_add(t1, t1, rot)          # t1 = roped q
                ms4 = smp.tile([128, HG], f32, tag="ms4")
                nc.vector.tensor_mul(rot, t1, t1)  # rot dead; reuse as q^2
                nc.vector.tensor_reduce(
                    ms4, rot.rearrange("p (h d) -> p h d", h=HG),
                    axis=mybir.AxisListType.X, op=add)
                srt4 = smp.tile([128, HG], f32, tag="srt4")
                nc.scalar.activation(out=srt4, in_=ms4, func=Sqrt,
                                     bias=eps_t[:, 0:1], scale=1.0 / D)
                nc.vector.reciprocal(srt4, srt4)
                for h in range(HG):
                    qn = qnp.tile([128, 128], f32, tag="qn")
                    nc.vector.tensor_scalar_mul(
                        qn, t1[:, h * 128:(h + 1) * 128], srt4[:, h:h + 1])
                    tp = pB.tile([128, DG], f32, tag="B")
                    nc.tensor.transpose(tp[:, 0:128], qn, ident)
                    nc.vector.tensor_copy(
                        qT_all[:, h, st * 128:(st + 1) * 128], tp[:, 0:128])

                # ---- RoPE + RMS-norm for k (single kv head) ----
                kk = kvp[:, 0, 0:D]
                k2 = kk.rearrange("p (t d) -> p t d", t=2)
                krot = rsp.tile([128, 128], f32, tag="krot")
                kr2 = krot.rearrange("p (t d) -> p t d", t=2)
                nc.vector.tensor_copy(kr2[:, 0, :], k2[:, 1, :])
                nc.vector.tensor_scalar_mul(kr2[:, 1, :], k2[:, 0, :], -1.0)
                kt1 = rsp.tile([128, 128], f32, tag="kt1")
                nc.vector.tensor_mul(kt1, kk, cs_t[:, 0, :])
                nc.vector.tensor_mul(krot, krot, sn_t[:, 0, :])
                nc.vector.tensor_add(kt1, kt1, krot)
                msk = smp.tile([128, 1], f32, tag="msk")
                nc.vector.tensor_mul(krot, kt1, kt1)  # krot dead; reuse as k^2
                nc.vector.tensor_reduce(msk, krot, axis=mybir.AxisListType.X, op=add)
                srtk = smp.tile([128, 1], f32, tag="srtk")
                nc.scalar.activation(out=srtk, in_=msk, func=Sqrt,
                                     bias=eps_t[:, 0:1], scale=1.0 / D)
                nc.vector.reciprocal(srtk, srtk)
                kn = qnp.tile([128, 128], f32, tag="kn")
                nc.vector.tensor_scalar_mul(kn, kt1, srtk)
                tp = pB.tile([128, DG], f32, tag="B")
                nc.tensor.transpose(tp[:, 0:128], kn, ident)
                nc.vector.tensor_copy(
                    kT_sb[:, st * 128:(st + 1) * 128], tp[:, 0:128])

            # ================= phase 2: attention + output projection =========
            for qc in range(PH2_CHUNKS):
                otile = otp.tile([128, HG, 512], f32r, tag="ot")  # [d, h, q]
                for h in range(HG):
                    if not EMIT_SCORES:
                        break
                    nkt = 4 * (qc + 1)
                    qrhs = qT_all[:, h, qc * 512:(qc + 1) * 512]
                    pts = []
                    for j2 in range(0, nkt, 2):
                        sp = pA.tile([128, 2, DG], f32, tag="A")
                        for j in range(2):
                            kt = j2 + j
                            nc.tensor.matmul(
                                sp[:, j, :],
                                lhsT=kT_sb[:, kt * 128:(kt + 1) * 128],
                                rhs=qrhs, start=True, stop=True)
                        pt = ptp.tile([128, 2, DG], bf16, tag="pt")
                        nc.scalar.activation(out=pt, in_=sp, func=Exp, scale=ISD)
                        pts.append(pt)
                    # mask the 4 diagonal (k_tile == q_tile) blocks
                    for qtl in range(4):
                        kt = 4 * qc + qtl
                        sl = pts[kt // 2][:, kt % 2, qtl * 128:(qtl + 1) * 128]
                        nc.vector.tensor_mul(sl, sl, cmask)
                    # probs @ [v | ones]
                    for qtl in range(4 if EMIT_PV else 0):
                        qt = 4 * qc + qtl
                        op = pC.tile([128, 132], f32, tag="C")
                        for kt in range(qt + 1):
                            nc.tensor.matmul(
                                op[:, 0:129],
                                lhsT=pts[kt // 2][:, kt % 2, qtl * 128:(qtl + 1) * 128],
                                rhs=vv[:, kt, 0:129],
                                start=(kt == 0), stop=(kt == qt))
                        rc = smp.tile([128, 1], f32, tag="rc")
                        nc.vector.reciprocal(rc, op[:, 128:129])
                        on = qnp.tile([128, 128], f32, tag="on")
                        nc.vector.tensor_scalar_mul(on, op[:, 0:128], rc)
                        tp = pB.tile([128, DG], f32, tag="B")
                        nc.tensor.transpose(tp[:, 0:128], on, ident)
                        nc.vector.tensor_copy(
                            otile[:, h, qtl * 128:(qtl + 1) * 128], tp[:, 0:128])
                # output projection for this chunk's 4 row tiles
                for stl in range(4 if EMIT_WO else 0):
                    srow = (4 * qc + stl) * 128
                    for cc in range(4):
                        wop = pB.tile([128, DG], f32, tag="B")
                        for h2 in range(HG):
                            nc.tensor.matmul(
                                wop,
                                lhsT=otile[:, h2, stl * 128:(stl + 1) * 128],
                                rhs=wo_sb[:, h2, cc * 512:(cc + 1) * 512],
                                start=(h2 == 0), stop=(h2 == HG - 1))
                        oc = ocp.tile([128, DG], f32, tag="oc")
                        nc.vector.tensor_copy(oc, wop)
                        nc.sync.dma_start(
                            out[srow:srow + 128, cc * 512:(cc + 1) * 512], oc)


def _build():
    nc = bacc.Bacc("TRN2", target_bir_lowering=False, debug=False,
                   num_devices=NCORES)
    xT = nc.dram_tensor("xT", [HID, S], f32r, kind="ExternalInput").ap()
    wqT = nc.dram_tensor("wqT", [HID, DG], f32r, kind="ExternalInput").ap()
    wkvT = nc.dram_tensor("wkvT", [HID, 2 * D], f32r, kind="ExternalInput").ap()
    woT = nc.dram_tensor("woT", [DG, HID], f32r, kind="ExternalInput").ap()
    csx = nc.dram_tensor("csx", [S, 128], f32, kind="ExternalInput").ap()
    snx = nc.dram_tensor("snx", [S, 128], f32, kind="ExternalInput").ap()
    out = nc.dram_tensor("out", [S, HID], f32, kind="ExternalOutput").ap()
    _emit(nc, xT, wqT, wkvT, woT, csx, snx, out)
    nc.compile()
    return nc


def _get_compiled():
    global _compiled
    if _compiled is None:
        _compiled = _build()
    return _compiled


def kernel(x, cos, sin, wq, wk, wv, wo):
    nc = _get_compiled()
    x = np.asarray(x, np.float32)
    cos = np.asarray(cos, np.float32)
    sin = np.asarray(sin, np.float32)
    wq = np.asarray(wq, np.float32)
    wk = np.asarray(wk, np.float32)
    wv = np.asarray(wv, np.float32)
    wo = np.asarray(wo, np.float32)

    wkvT = np.ascontiguousarray(np.concatenate([wk, wv], 0).T)
    csx = np.ascontiguousarray(np.concatenate([cos, cos], 1))
    snx = np.ascontiguousarray(np.concatenate([sin, sin], 1))
    xTs = [np.ascontiguousarray(x[b].T) for b in range(B)]
    wqTs = [np.ascontiguousarray(wq[g * DG:(g + 1) * DG].T) for g in range(GROUPS)]
    woTs = [np.ascontiguousarray(wo[:, g * DG:(g + 1) * DG].T) for g in range(GROUPS)]

    in_maps = []
    for c in range(NCORES):
        b, g = divmod(c, GROUPS)
        in_maps.append({
            "xT": xTs[b], "wqT": wqTs[g], "wkvT": wkvT, "woT": woTs[g],
            "csx": csx, "snx": snx,
        })
    res = run_bass_kernel_spmd(nc, in_maps, list(range(NCORES)), trace=TRACE)
    LAST["res"] = res
    outs = [r["out"] for r in res.results]
    final = np.empty((B, S, HID), np.float32)
    for b in range(B):
        final[b] = (outs[GROUPS * b] + outs[GROUPS * b + 1]
                    + outs[GROUPS * b + 2] + outs[GROUPS * b + 3])
    return final


# revision 9
# speedup vs baseline: 1.0892x; 1.0892x over previous
"""Causal MQA self-attention (RoPE + RMS-norm on q/k) on 8 TRN2 NeuronCores.

Sharding: core c -> (batch b = c//4, head-group g = c%4 of 4 heads).
Each core computes, for its batch and its 4 heads:
  q/k/v projections -> RoPE -> RMS-norm -> causal attention -> partial
  output projection out_part = attn_out_g @ wo[:, g].T  (shape [S, HID]).
Host sums the 4 per-group partials of each batch (row-parallel matmul
unshard) and stacks the 2 batches.

All heavy matmuls run in float32r (full PE rate at moving-dim >= 256,
~1.6e-4 scale-relative accuracy). Attention probabilities and V run in
bf16 with fp32 PSUM accumulation; the softmax denominator comes free via
a ones-column appended to V. No max-subtraction is needed: post-RMS-norm
|q|=|k|=sqrt(D), so scores <= sqrt(D) ~ 11.3 and exp stays in range.
"""

import ml_dtypes
import numpy as np

import concourse.bass as bass
import concourse.mybir as mybir
import concourse.tile as tile
from concourse import bacc
from concourse.bass_utils import run_bass_kernel_spmd
from concourse.masks import make_identity, make_upper_triangular

# problem dims (hardcoded per contract)
B, S, HID, H, D = 2, 2048, 2048, 16, 128
NCORES = 8
GROUPS = 4              # head-groups = cores per batch
HG = H // GROUPS        # heads per core
DG = HG * D             # 512 projected q dims per core
NT = S // 128           # 16 sequence tiles
HT = HID // 128         # 16 hidden tiles
NQC = 4                 # q chunks of 512 columns
EPS = 1.1920928955078125e-07
ISD = 1.0 / float(np.sqrt(D))

f32 = mybir.dt.float32
f32r = mybir.dt.float32r
bf16 = mybir.dt.bfloat16

TRACE = False           # test harness may flip this for NTFF profiling
PH1_TILES = NT          # bisect knob: how many s-tiles to emit in phase 1
PH2_CHUNKS = NQC        # bisect knob: how many q-chunks to emit in phase 2
EMIT_SCORES = True      # bisect knob
EMIT_PV = True          # bisect knob
EMIT_WO = True          # bisect knob
LAST = {}               # last BassKernelResults, for the test harness

_compiled = None


def _emit(nc, xT, wqT, wkvT, woT, csx, snx, out):
    mult = mybir.AluOpType.mult
    add = mybir.AluOpType.add
    Sqrt = mybir.ActivationFunctionType.Sqrt
    Exp = mybir.ActivationFunctionType.Exp

    with tile.TileContext(nc) as tc:
        with (
            tc.tile_pool(name="consts", bufs=1) as consts,
            tc.tile_pool(name="bigp", bufs=1) as bigp,
            tc.tile_pool(name="xsp", bufs=3) as xsp,
            tc.tile_pool(name="csp", bufs=1) as csp,
            tc.tile_pool(name="rsp", bufs=2) as rsp,
            tc.tile_pool(name="smp", bufs=2) as smp,
            tc.tile_pool(name="qnp", bufs=2) as qnp,
            tc.tile_pool(name="ptp", bufs=9) as ptp,
            tc.tile_pool(name="otp", bufs=2) as otp,
            tc.tile_pool(name="ocp", bufs=2) as ocp,
            tc.tile_pool(name="pA", bufs=2, space="PSUM") as pA,
            tc.tile_pool(name="pB", bufs=2, space="PSUM") as pB,
            tc.tile_pool(name="pC", bufs=2, space="PSUM") as pC,
        ):
            # ---- constants ----
            ident = consts.tile([128, 128], bf16)
            make_identity(nc, ident)
            cmask = consts.tile([128, 128], bf16)  # 1 where k <= q else 0
            make_upper_triangular(nc, cmask, val=1.0, diag=True)
            eps_t = consts.tile([128, 1], f32)
            nc.vector.memset(eps_t, EPS)

            # ---- resident weights / activations ----
            wq_sb = bigp.tile([128, HT, DG], bf16, tag="wq")
            nc.sync.dma_start(wq_sb, wqT.rearrange("(t p) d -> p t d", p=128))
            wkv_sb = bigp.tile([128, HT, 2 * D], bf16, tag="wkv")
            nc.sync.dma_start(wkv_sb, wkvT.rearrange("(t p) d -> p t d", p=128))
            wo_sb = bigp.tile([128, HG, HID], bf16, tag="wo")
            nc.sync.dma_start(wo_sb, woT.rearrange("(h p) n -> p h n", p=128))

            qT_all = bigp.tile([128, HG, S], bf16, tag="qT")   # [d, h, s]
            kT_sb = bigp.tile([128, S], bf16, tag="kT")        # [d, s]
            vv = bigp.tile([128, NT, 132], bf16, tag="vv")     # [s%128, s//128, d+ones]
            nc.vector.memset(vv[:, :, 128:132], 1.0)

            xTr = xT.rearrange("(t p) s -> p t s", p=128)

            def bcast4(src2d, st):
                base = src2d[st * 128:(st + 1) * 128, :]
                return bass.AP(
                    tensor=base.tensor,
                    offset=base.offset,
                    ap=[base.ap[0], [0, HG], base.ap[1]],
                )

            # ================= phase 1: projections + RoPE + RMS-norm =========
            for st in range(PH1_TILES):
                xs0 = xsp.tile([128, HT // 2, 128], bf16, tag="xs")
                nc.sync.dma_start(xs0, xTr[:, 0:HT // 2, st * 128:(st + 1) * 128])
                xs1 = xsp.tile([128, HT // 2, 128], bf16, tag="xs")
                nc.sync.dma_start(xs1, xTr[:, HT // 2:HT, st * 128:(st + 1) * 128])
                xhalves = (xs0, xs1)

                cs_t = csp.tile([128, HG, 128], f32, tag="cs")
                nc.gpsimd.dma_start(cs_t, bcast4(csx, st))
                sn_t = csp.tile([128, HG, 128], f32, tag="sn")
                nc.gpsimd.dma_start(sn_t, bcast4(snx, st))

                qp = pA.tile([128, 2, DG], f32, tag="A")
                for t in range(HT):
                    nc.tensor.matmul(
                        qp[:, 0, :], lhsT=xhalves[t // 8][:, t % 8, :],
                        rhs=wq_sb[:, t, :], start=(t == 0), stop=(t == HT - 1),
                    )
                kvp = pA.tile([128, 2, DG], f32, tag="A")
                for t in range(HT):
                    nc.tensor.matmul(
                        kvp[:, 0, 0:2 * D], lhsT=xhalves[t // 8][:, t % 8, :],
                        rhs=wkv_sb[:, t, :], start=(t == 0), stop=(t == HT - 1),
                    )

                # v -> bf16 tiles (ones column preset)
                nc.vector.tensor_copy(vv[:, st, 0:128], kvp[:, 0, D:2 * D])

                # ---- RoPE + RMS-norm for 4 q heads, batched ----
                qflat = qp[:, 0, :]
                q3 = qflat.rearrange("p (h d) -> p h d", h=HG)
                q4 = qflat.rearrange("p (h t d) -> p h t d", h=HG, t=2)
                rot = rsp.tile([128, DG], f32, tag="rot")
                r4 = rot.rearrange("p (h t d) -> p h t d", h=HG, t=2)
                r3 = rot.rearrange("p (h d) -> p h d", h=HG)
                nc.vector.tensor_copy(r4[:, :, 0, :], q4[:, :, 1, :])
                nc.vector.tensor_scalar_mul(r4[:, :, 1, :], q4[:, :, 0, :], -1.0)
                t1 = rsp.tile([128, DG], f32, tag="t1")
                t3 = t1.rearrange("p (h d) -> p h d", h=HG)
                nc.vector.tensor_mul(t3, q3, cs_t)
                nc.vector.tensor_mul(r3, r3, sn_t)
                nc.vector.tensor_add(t1, t1, rot)          # t1 = roped q
                ms4 = smp.tile([128, HG], f32, tag="ms4")
                nc.vector.tensor_mul(rot, t1, t1)  # rot dead; reuse as q^2
                nc.vector.tensor_reduce(
                    ms4, rot.rearrange("p (h d) -> p h d", h=HG),
                    axis=mybir.AxisListType.X, op=add)
                srt4 = smp.tile([128, HG], f32, tag="srt4")
                nc.scalar.activation(out=srt4, in_=ms4, func=Sqrt,
                                     bias=eps_t[:, 0:1], scale=1.0 / D)
                nc.vector.reciprocal(srt4, srt4)
                for h in range(HG):
                    qn = qnp.tile([128, 128], bf16, tag="qn")
                    nc.vector.tensor_scalar_mul(
                        qn, t1[:, h * 128:(h + 1) * 128], srt4[:, h:h + 1])
                    tp = pB.tile([128, DG], f32, tag="B")
                    nc.tensor.transpose(tp[:, 0:64].bitcast(bf16), qn, ident)
                    nc.vector.tensor_copy(
                        qT_all[:, h, st * 128:(st + 1) * 128],
                        tp[:, 0:64].bitcast(bf16))

                # ---- RoPE + RMS-norm for k (single kv head) ----
                kk = kvp[:, 0, 0:D]
                k2 = kk.rearrange("p (t d) -> p t d", t=2)
                krot = rsp.tile([128, 128], f32, tag="krot")
                kr2 = krot.rearrange("p (t d) -> p t d", t=2)
                nc.vector.tensor_copy(kr2[:, 0, :], k2[:, 1, :])
                nc.vector.tensor_scalar_mul(kr2[:, 1, :], k2[:, 0, :], -1.0)
                kt1 = rsp.tile([128, 128], f32, tag="kt1")
                nc.vector.tensor_mul(kt1, kk, cs_t[:, 0, :])
                nc.vector.tensor_mul(krot, krot, sn_t[:, 0, :])
                nc.vector.tensor_add(kt1, kt1, krot)
                msk = smp.tile([128, 1], f32, tag="msk")
                nc.vector.tensor_mul(krot, kt1, kt1)  # krot dead; reuse as k^2
                nc.vector.tensor_reduce(msk, krot, axis=mybir.AxisListType.X, op=add)
                srtk = smp.tile([128, 1], f32, tag="srtk")
                nc.scalar.activation(out=srtk, in_=msk, func=Sqrt,
                                     bias=eps_t[:, 0:1], scale=1.0 / D)
                nc.vector.reciprocal(srtk, srtk)
                kn = qnp.tile([128, 128], bf16, tag="kn")
                nc.vector.tensor_scalar_mul(kn, kt1, srtk)
                tp = pB.tile([128, DG], f32, tag="B")
                nc.tensor.transpose(tp[:, 0:64].bitcast(bf16), kn, ident)
                nc.vector.tensor_copy(
                    kT_sb[:, st * 128:(st + 1) * 128],
                    tp[:, 0:64].bitcast(bf16))

            # ================= phase 2: attention + output projection =========
            for qc in range(PH2_CHUNKS):
                otile = otp.tile([128, HG, 512], bf16, tag="ot")  # [d, h, q]
                for h in range(HG):
                    if not EMIT_SCORES:
                        break
                    nkt = 4 * (qc + 1)
                    qrhs = qT_all[:, h, qc * 512:(qc + 1) * 512]
                    pts = []
                    for j2 in range(0, nkt, 2):
                        sp = pA.tile([128, 2, DG], f32, tag="A")
                        for j in range(2):
                            kt = j2 + j
                            nc.tensor.matmul(
                                sp[:, j, :],
                                lhsT=kT_sb[:, kt * 128:(kt + 1) * 128],
                                rhs=qrhs, start=True, stop=True)
                        pt = ptp.tile([128, 2, DG], bf16, tag="pt")
                        nc.scalar.activation(out=pt, in_=sp, func=Exp, scale=ISD)
                        pts.append(pt)
                    # mask the 4 diagonal (k_tile == q_tile) blocks
                    for qtl in range(4):
                        kt = 4 * qc + qtl
                        sl = pts[kt // 2][:, kt % 2, qtl * 128:(qtl + 1) * 128]
                        nc.vector.tensor_mul(sl, sl, cmask)
                    # probs @ [v | ones]
                    for qtl in range(4 if EMIT_PV else 0):
                        qt = 4 * qc + qtl
                        op = pC.tile([128, 132], f32, tag="C")
                        for kt in range(qt + 1):
                            nc.tensor.matmul(
                                op[:, 0:129],
                                lhsT=pts[kt // 2][:, kt % 2, qtl * 128:(qtl + 1) * 128],
                                rhs=vv[:, kt, 0:129],
                                start=(kt == 0), stop=(kt == qt))
                        rc = smp.tile([128, 1], f32, tag="rc")
                        nc.vector.reciprocal(rc, op[:, 128:129])
                        on = qnp.tile([128, 128], bf16, tag="on")
                        nc.vector.tensor_scalar_mul(on, op[:, 0:128], rc)
                        tp = pB.tile([128, DG], f32, tag="B")
                        nc.tensor.transpose(tp[:, 0:64].bitcast(bf16), on, ident)
                        nc.vector.tensor_copy(
                            otile[:, h, qtl * 128:(qtl + 1) * 128],
                            tp[:, 0:64].bitcast(bf16))
                # output projection for this chunk's 4 row tiles
                for stl in range(4 if EMIT_WO else 0):
                    srow = (4 * qc + stl) * 128
                    for cc in range(4):
                        wop = pB.tile([128, DG], f32, tag="B")
                        for h2 in range(HG):
                            nc.tensor.matmul(
                                wop,
                                lhsT=otile[:, h2, stl * 128:(stl + 1) * 128],
                                rhs=wo_sb[:, h2, cc * 512:(cc + 1) * 512],
                                start=(h2 == 0), stop=(h2 == HG - 1))
                        oc = ocp.tile([128, DG], f32, tag="oc")
                        nc.vector.tensor_copy(oc, wop)
                        nc.sync.dma_start(
                            out[srow:srow + 128, cc * 512:(cc + 1) * 512], oc)


def _build():
    nc = bacc.Bacc("TRN2", target_bir_lowering=False, debug=False,
                   num_devices=NCORES)
    xT = nc.dram_tensor("xT", [HID, S], bf16, kind="ExternalInput").ap()
    wqT = nc.dram_tensor("wqT", [HID, DG], bf16, kind="ExternalInput").ap()
    wkvT = nc.dram_tensor("wkvT", [HID, 2 * D], bf16, kind="ExternalInput").ap()
    woT = nc.dram_tensor("woT", [DG, HID], bf16, kind="ExternalInput").ap()
    csx = nc.dram_tensor("csx", [S, 128], f32, kind="ExternalInput").ap()
    snx = nc.dram_tensor("snx", [S, 128], f32, kind="ExternalInput").ap()
    out = nc.dram_tensor("out", [S, HID], f32, kind="ExternalOutput").ap()
    _emit(nc, xT, wqT, wkvT, woT, csx, snx, out)
    nc.compile()
    return nc


def _get_compiled():
    global _compiled
    if _compiled is None:
        _compiled = _build()
    return _compiled


def kernel(x, cos, sin, wq, wk, wv, wo):
    nc = _get_compiled()
    x = np.asarray(x, np.float32)
    cos = np.asarray(cos, np.float32)
    sin = np.asarray(sin, np.float32)
    wq = np.asarray(wq, np.float32)
    wk = np.asarray(wk, np.float32)
    wv = np.asarray(wv, np.float32)
    wo = np.asarray(wo, np.float32)

    bf = ml_dtypes.bfloat16
    wkvT = np.ascontiguousarray(np.concatenate([wk, wv], 0).T.astype(bf))
    csx = np.ascontiguousarray(np.concatenate([cos, cos], 1))
    snx = np.ascontiguousarray(np.concatenate([sin, sin], 1))
    xTs = [np.ascontiguousarray(x[b].T.astype(bf)) for b in range(B)]
    wqTs = [np.ascontiguousarray(wq[g * DG:(g + 1) * DG].T.astype(bf)) for g in range(GROUPS)]
    woTs = [np.ascontiguousarray(wo[:, g * DG:(g + 1) * DG].T.astype(bf)) for g in range(GROUPS)]

    in_maps = []
    for c in range(NCORES):
        b, g = divmod(c, GROUPS)
        in_maps.append({
            "xT": xTs[b], "wqT": wqTs[g], "wkvT": wkvT, "woT": woTs[g],
            "csx": csx, "snx": snx,
        })
    res = run_bass_kernel_spmd(nc, in_maps, list(range(NCORES)), trace=TRACE)
    LAST["res"] = res
    outs = [r["out"] for r in res.results]
    final = np.empty((B, S, HID), np.float32)
    for b in range(B):
        final[b] = (outs[GROUPS * b] + outs[GROUPS * b + 1]
                    + outs[GROUPS * b + 2] + outs[GROUPS * b + 3])
    return final
